# revision 11
# baseline (speedup 1.0000x reference)
"""CLAM attention-pooling kernel for 8 Trainium2 NeuronCores.

Model (reference):
    h = relu(x @ Wc + bc)                      [N, 512]
    a = tanh(x @ W1 + b1) * sigmoid(x @ W2 + b2)
    A = a @ W3 + b3                            [N, 2]
    slide_agg = A.T @ h                        [2, 512]
    score / softmax / argmax;  top-k(+/-) rows of A[:, label] -> ins_logits

Sharding: the instance dim N=100000 is split row-wise across 8 cores
(12500 rows each).  Each core makes a single bf16 pass over its x shard:
    - transpose x tiles on TensorE (matmul against identity)
    - fused matmul x @ [Wc | W1 | W2]  (K=1024 accumulated in PSUM)
    - Relu/Tanh/Sigmoid on ScalarE, g = t*s on VectorE
    - A = g @ W3 via two more small transposes
    - running A^T @ h accumulated in a persistent PSUM bank
      (with an extra ones-column so sum_r h[r] comes out too)
Per-core outputs: agg-partial [3, 512] and the full A rows [128, 98, 2].
The host sums partials, does the tiny slide head exactly, shortlists
top-k candidates from the device A column, and rescores the shortlist
(plus the 16 selected h rows) exactly in float64.
"""

import numpy as np
import ml_dtypes

N, D, DC, H, C = 100000, 1024, 512, 256, 2
NCORES = 8
NSH = N // NCORES            # 12500 rows per core
P = 128
DOUT = DC + 2 * H            # 1024 fused output columns
KC = D // P                  # 8 contraction chunks

_cache = {}


def _build_program(n_rows, n_iters, with_bias):
    """Build the per-core Bass program (SPMD; same program on all cores)."""
    import concourse.bass as bass
    import concourse.mybir as mybir
    import concourse.tile as tile
    from concourse import bacc
    from concourse.masks import make_identity

    f32 = mybir.dt.float32
    bf16 = mybir.dt.bfloat16

    n_tiles = (n_rows + P - 1) // P
    af = mybir.ActivationFunctionType

    nc = bacc.Bacc(None, target_bir_lowering=False, debug=False)
    x_ext = nc.declare_dram_parameter("x", [n_rows, D], f32, isOutput=False)
    w_ext = nc.declare_dram_parameter("wcat", [D, DOUT], bf16, isOutput=False)
    w3_ext = nc.declare_dram_parameter("w3", [H, C], bf16, isOutput=False)
    if with_bias:
        b_ext = nc.declare_dram_parameter("bcat", [1, DOUT], f32, isOutput=False)
    agg_ext = nc.declare_dram_parameter("agg", [C + 1, DC], f32, isOutput=True)
    acol_ext = nc.declare_dram_parameter("acol", [P, n_tiles, C], f32, isOutput=True)

    with tile.TileContext(nc) as tc:
        with (
            tc.tile_pool(name="singles", bufs=1) as singles,
            tc.tile_pool(name="xb", bufs=4) as xb_pool,
            tc.tile_pool(name="xtb", bufs=3) as xtb_pool,
            tc.tile_pool(name="acts", bufs=3) as acts_pool,
            tc.tile_pool(name="small", bufs=4) as small_pool,
            tc.tile_pool(name="ps_out", bufs=3, space="PSUM") as ps_out,
            tc.tile_pool(name="ps_agg", bufs=1, space="PSUM") as ps_agg,
        ):
            # ---- constants / weights (loaded once) ----
            w_sb = singles.tile([P, KC, DOUT], bf16)
            nc.sync.dma_start(
                out=w_sb, in_=w_ext.ap().rearrange("(c p) n -> p c n", p=P)
            )
            # W3 replicated across partitions: w3rep[p, c, j] = W3[j, c]
            w3rep = singles.tile([P, C, H], bf16)
            for c in range(C):
                nc.sync.dma_start(
                    out=w3rep[:, c, :],
                    in_=bass.AP(
                        tensor=w3_ext.ap().tensor,
                        offset=c,
                        ap=[[0, P], [C, H]],
                    ),
                )
            if with_bias:
                bias_sb = singles.tile([1, DOUT], f32)
                nc.sync.dma_start(out=bias_sb, in_=b_ext.ap())
                ones_row = singles.tile([1, P], bf16)
                nc.vector.memset(ones_row, 1.0)
            stage = singles.tile([P, n_tiles, C], f32)
            agg_ps = ps_agg.tile([C + 1, DC], f32)

            def emit_agg(pending):
                a_sb, h_sb, rows, t = pending
                nc.tensor.matmul(
                    agg_ps,
                    lhsT=a_sb[:rows],
                    rhs=h_sb[:rows],
                    start=(t == 0),
                    stop=(t == n_tiles - 1),
                )

            def body():
                pending_agg = None
                for t in range(n_tiles):
                    rows = min(P, n_rows - t * P)
                    # -- load x tile, f32->bf16 cast done by the DMA engine --
                    xb = xb_pool.tile([P, D], bf16, tag="xb")
                    nc.gpsimd.dma_start(
                        out=xb[:rows], in_=x_ext.ap()[t * P : t * P + rows, :]
                    )

                    # -- transpose x tile: 8 chunks of [128,128] via the DMA
                    #    xbar transpose engine (SBUF -> SBUF) --
                    xtb = xtb_pool.tile([P, KC, P], bf16, tag="xtb")
                    for k in range(KC):
                        nc.sync.dma_start(
                            out=xtb[:, k, :],
                            in_=xb[:, k * P : (k + 1) * P],
                            transpose=True,
                        )

                    # -- fused matmul: out = x_tile @ [Wc|W1|W2] --
                    out_ps = ps_out.tile([P, DOUT], f32, tag="out")
                    for j in range(2):
                        sl = slice(j * 512, (j + 1) * 512)
                        for k in range(KC):
                            nc.tensor.matmul(
                                out_ps[:rows, sl],
                                lhsT=xtb[:, k, :rows],
                                rhs=w_sb[:, k, sl],
                                start=(k == 0),
                                stop=(k == KC - 1) if not with_bias else False,
                            )
                        if with_bias:
                            nc.tensor.matmul(
                                out_ps[:rows, sl],
                                lhsT=ones_row[0:1, :rows],
                                rhs=bias_sb[0:1, sl],
                                start=False,
                                stop=True,
                            )

                    # -- deferred agg for the PREVIOUS tile: its a_sb/h_sb had
                    #    a whole tile's worth of TE time to become ready, so
                    #    TensorE never stalls on the ACT/DVE epilogue --
                    if pending_agg is not None:
                        emit_agg(pending_agg)

                    # -- activations --
                    h_sb = acts_pool.tile([P, DC], bf16, tag="h")
                    nc.scalar.activation(
                        out=h_sb[:rows], in_=out_ps[:rows, 0:DC], func=af.Relu
                    )
                    t_sb = acts_pool.tile([P, H], bf16, tag="t")
                    nc.scalar.activation(
                        out=t_sb[:rows], in_=out_ps[:rows, DC : DC + H], func=af.Tanh
                    )
                    s_sb = acts_pool.tile([P, H], bf16, tag="s")
                    nc.scalar.activation(
                        out=s_sb[:rows],
                        in_=out_ps[:rows, DC + H : DOUT],
                        func=af.Sigmoid,
                    )
                    g_sb = acts_pool.tile([P, H], bf16, tag="g")
                    nc.vector.tensor_mul(g_sb[:rows], t_sb[:rows], s_sb[:rows])

                    # -- A[:, c] = sum_j g[:, j] * W3[j, c]  on DVE --
                    a_f32 = small_pool.tile([P, C], f32, tag="af32")
                    ttr_scr = small_pool.tile([P, H], f32, tag="ttrscr")
                    for c in range(C):
                        nc.vector.scalar_tensor_tensor(
                            out=ttr_scr[:rows],
                            in0=g_sb[:rows],
                            scalar=1.0,
                            in1=w3rep[:rows, c, :],
                            op0=mybir.AluOpType.mult,
                            op1=mybir.AluOpType.mult,
                            accum_out=a_f32[:rows, c : c + 1],
                        )

                    # -- stash A (both columns, f32) + bf16 copy for agg --
                    a_sb = small_pool.tile([P, C + 1], bf16, tag="asb")
                    nc.vector.tensor_copy(a_sb[:rows, 0:C], a_f32[:rows])
                    nc.vector.memset(a_sb[:rows, C : C + 1], 1.0)
                    nc.vector.tensor_copy(stage[:rows, t, :], a_f32[:rows])

                    # -- agg += [A | 1].T @ h, deferred one tile --
                    pending_agg = (a_sb, h_sb, rows, t)

                emit_agg(pending_agg)

                # -- write outputs --
                agg_sb = small_pool.tile([C + 1, DC], f32, tag="aggsb")
                nc.vector.tensor_copy(agg_sb, agg_ps)
                nc.sync.dma_start(out=agg_ext.ap(), in_=agg_sb)
                nc.sync.dma_start(out=acol_ext.ap(), in_=stage)

            if n_iters > 1:
                with tc.For_i(0, n_iters, 1):
                    body()
            else:
                body()

    nc.compile()
    return nc


def _get_program(n_rows=NSH, n_iters=1, with_bias=False):
    key = (n_rows, n_iters, with_bias)
    if key not in _cache:
        _cache[key] = _build_program(n_rows, n_iters, with_bias)
    return _cache[key]


def _softmax(v):
    v = v - v.max()
    e = np.exp(v)
    return e / e.sum()


def run_device(x, wcat32, W3, bcat32, n_rows=NSH, n_cores=NCORES, n_iters=1):
    """Run the SPMD program; returns (agg_partials [cores,3,512],
    A_cols [cores, P, n_tiles, C]) as float64."""
    from concourse.bass_utils import run_bass_kernel_spmd

    with_bias = bool(np.any(bcat32))
    nc = _get_program(n_rows, n_iters, with_bias)
    wcat_b = wcat32.astype(ml_dtypes.bfloat16)
    w3_b = W3.astype(ml_dtypes.bfloat16)
    in_maps = []
    for i in range(n_cores):
        m = {
            "x": np.ascontiguousarray(x[i * n_rows : (i + 1) * n_rows]),
            "wcat": wcat_b,
            "w3": w3_b,
        }
        if with_bias:
            m["bcat"] = bcat32.reshape(1, DOUT)
        in_maps.append(m)
    res = run_bass_kernel_spmd(nc, in_maps, list(range(n_cores))).results
    agg = np.stack([r["agg"] for r in res]).astype(np.float64)
    acol = np.stack([r["acol"] for r in res]).astype(np.float64)
    return agg, acol


def kernel(**inputs):
    x = np.asarray(inputs["x"], dtype=np.float32)
    Wc = np.asarray(inputs["Wc"], dtype=np.float32)
    bc = np.asarray(inputs["bc"], dtype=np.float32)
    W1 = np.asarray(inputs["W1"], dtype=np.float32)
    b1 = np.asarray(inputs["b1"], dtype=np.float32)
    W2 = np.asarray(inputs["W2"], dtype=np.float32)
    b2 = np.asarray(inputs["b2"], dtype=np.float32)
    W3 = np.asarray(inputs["W3"], dtype=np.float32)
    b3 = np.asarray(inputs["b3"], dtype=np.float32)
    Wbag = np.asarray(inputs["Wbag"], dtype=np.float32)
    bbag = np.asarray(inputs["bbag"], dtype=np.float32)
    Wins = np.asarray(inputs["Wins"], dtype=np.float32)
    bins = np.asarray(inputs["bins"], dtype=np.float32)
    label = int(inputs["bag_label"])
    n_ins = int(inputs["n_ins"])

    wcat = np.concatenate([Wc, W1, W2], axis=1)          # [1024, 1024]
    bcat = np.concatenate([bc, b1, b2])                  # [1024]

    agg_p, acol_p = run_device(x, wcat, W3, bcat)

    # ---- slide head (host, exact given device agg partials) ----
    agg_all = agg_p.sum(axis=0)                          # [3, 512]
    slide_agg = agg_all[:C] + np.outer(b3.astype(np.float64), agg_all[C])
    score = slide_agg @ Wbag.astype(np.float64) + bbag.astype(np.float64)
    score = score.reshape(1, -1)                         # [1, 2]
    Y_prob = _softmax(score[0]).reshape(1, -1)
    Y_hat = np.int32(np.argmax(score[0]))

    # ---- assemble device A column over all N rows ----
    n_tiles = (NSH + P - 1) // P
    # acol_p[c, p, t, :] is row  c*NSH + t*128 + p
    a_dev = acol_p[:, :, :, label].transpose(0, 2, 1).reshape(NCORES, n_tiles * P)
    a_dev = a_dev[:, :NSH].reshape(-1)                   # [100000]

    # ---- shortlist + exact rescore in float64 ----
    K = min(N, max(64, 8 * n_ins))
    cand_pos = np.argpartition(-a_dev, K - 1)[:K]
    cand_neg = np.argpartition(a_dev, K - 1)[:K]
    cands = np.unique(np.concatenate([cand_pos, cand_neg]))
    xr = x[cands].astype(np.float64)
    a_ex = np.tanh(xr @ W1 + b1) * (1.0 / (1.0 + np.exp(-(xr @ W2 + b2))))
    A_ex = a_ex @ W3[:, label].astype(np.float64) + np.float64(b3[label])
    order = np.argsort(-A_ex, kind="stable")
    pos_idx = cands[order[:n_ins]]
    neg_idx = cands[order[::-1][:n_ins]]
    sel = np.concatenate([pos_idx, neg_idx])

    xs = x[sel].astype(np.float64)
    h_sel = np.maximum(xs @ Wc + bc, 0.0)
    logits = h_sel @ Wins + bins
    ins_logits = np.stack([_softmax(row) for row in logits]).astype(np.float32)
    ins_labels = np.concatenate(
        [np.ones(n_ins, np.int32), np.zeros(n_ins, np.int32)]
    )

    return (
        score.astype(np.float32),
        Y_hat,
        Y_prob.astype(np.float32),
        ins_logits,
        ins_labels,
    )


# revision 13
# speedup vs baseline: 2.3378x; 2.3378x over previous
"""CLAM attention-pooling kernel for 8 Trainium2 NeuronCores.

Model (reference):
    h = relu(x @ Wc + bc)                      [N, 512]
    a = tanh(x @ W1 + b1) * sigmoid(x @ W2 + b2)
    A = a @ W3 + b3                            [N, 2]
    slide_agg = A.T @ h                        [2, 512]
    score / softmax / argmax;  top-k(+/-) rows of A[:, label] -> ins_logits

Sharding: the instance dim N=100000 is split row-wise across 8 cores
(12500 rows each).  Each core makes a single bf16 pass over its x shard:
    - f32->bf16 cast done in-flight by the (SWDGE) DMA engines
    - transpose x tiles on TensorE (matmul against identity)
    - fused matmul x @ [Wc | W1 | W2]  (K=1024 accumulated in PSUM)
    - Relu/Tanh/Sigmoid on ScalarE, g = t*s on VectorE
    - A = g @ W3 as a broadcast multiply-reduce on VectorE
    - running A^T @ h accumulated in a persistent PSUM bank
      (with an extra ones-column so sum_r h[r] comes out too); each
      tile's agg matmul is deferred one tile so TensorE never stalls
      waiting for the ScalarE/VectorE epilogue
Per-core outputs: agg-partial [3, 512] and the full A rows [128, 98, 2].
The host sums partials, does the tiny slide head exactly, shortlists
top-k candidates from the device A column, and rescores the shortlist
(plus the 16 selected h rows) exactly in float64.
"""

import numpy as np
import ml_dtypes

N, D, DC, H, C = 100000, 1024, 512, 256, 2
NCORES = 8
NSH = N // NCORES            # 12500 rows per core
P = 128
DOUT = DC + 2 * H            # 1024 fused output columns
KC = D // P                  # 8 contraction chunks

_cache = {}


def _build_program(n_rows, n_iters, with_bias):
    """Build the per-core Bass program (SPMD; same program on all cores)."""
    import concourse.bass as bass
    import concourse.mybir as mybir
    import concourse.tile as tile
    from concourse import bacc
    from concourse.masks import make_identity

    f32 = mybir.dt.float32
    bf16 = mybir.dt.bfloat16

    n_tiles = (n_rows + P - 1) // P
    af = mybir.ActivationFunctionType

    nc = bacc.Bacc(None, target_bir_lowering=False, debug=False)
    x_ext = nc.declare_dram_parameter("x", [n_rows, D], f32, isOutput=False)
    w_ext = nc.declare_dram_parameter("wcat", [D, DOUT], bf16, isOutput=False)
    w3_ext = nc.declare_dram_parameter("w3", [H, C], bf16, isOutput=False)
    if with_bias:
        b_ext = nc.declare_dram_parameter("bcat", [1, DOUT], f32, isOutput=False)
    agg_ext = nc.declare_dram_parameter("agg", [C + 1, DC], f32, isOutput=True)
    acol_ext = nc.declare_dram_parameter("acol", [P, n_tiles, C], f32, isOutput=True)

    with tile.TileContext(nc) as tc:
        with (
            tc.tile_pool(name="singles", bufs=1) as singles,
            tc.tile_pool(name="xb", bufs=4) as xb_pool,
            tc.tile_pool(name="xtb", bufs=3) as xtb_pool,
            tc.tile_pool(name="acts", bufs=3) as acts_pool,
            tc.tile_pool(name="small", bufs=4) as small_pool,
            tc.tile_pool(name="ps_small", bufs=3, space="PSUM") as ps_small,
            tc.tile_pool(name="ps_out", bufs=2, space="PSUM") as ps_out,
            tc.tile_pool(name="ps_agg", bufs=1, space="PSUM") as ps_agg,
        ):
            # ---- constants / weights (loaded once) ----
            ident = singles.tile([P, P], bf16)
            make_identity(nc, ident)
            w_sb = singles.tile([P, KC, DOUT], bf16)
            nc.sync.dma_start(
                out=w_sb, in_=w_ext.ap().rearrange("(c p) n -> p c n", p=P)
            )
            # W3 replicated across partitions: w3rep[p, c, j] = W3[j, c]
            w3rep = singles.tile([P, C, H], bf16)
            for c in range(C):
                nc.sync.dma_start(
                    out=w3rep[:, c, :],
                    in_=bass.AP(
                        tensor=w3_ext.ap().tensor,
                        offset=c,
                        ap=[[0, P], [C, H]],
                    ),
                )
            if with_bias:
                bias_sb = singles.tile([1, DOUT], f32)
                nc.sync.dma_start(out=bias_sb, in_=b_ext.ap())
                ones_row = singles.tile([1, P], bf16)
                nc.vector.memset(ones_row, 1.0)
            stage = singles.tile([P, n_tiles, C], f32)
            agg_ps = ps_agg.tile([C + 1, DC], f32)

            def emit_agg(pending):
                a_sb, h_sb, rows, t = pending
                nc.tensor.matmul(
                    agg_ps,
                    lhsT=a_sb[:rows],
                    rhs=h_sb[:rows],
                    start=(t == 0),
                    stop=(t == n_tiles - 1),
                )

            def body():
                pending_agg = None
                for t in range(n_tiles):
                    rows = min(P, n_rows - t * P)
                    # -- load x tile, f32->bf16 cast done by the DMA engine --
                    xb = xb_pool.tile([P, D], bf16, tag="xb")
                    nc.gpsimd.dma_start(
                        out=xb[:rows], in_=x_ext.ap()[t * P : t * P + rows, :]
                    )

                    # -- transpose x tile: 8 chunks of [128,128] via matmul
                    #    against identity, 4 chunks per PSUM bank --
                    xtb = xtb_pool.tile([P, D], bf16, tag="xtb")
                    for half in range(2):
                        t_ps = ps_small.tile([P, 4 * P], f32, tag="pss")
                        for kk in range(4):
                            k = half * 4 + kk
                            nc.tensor.matmul(
                                t_ps[:, kk * P : kk * P + rows],
                                lhsT=xb[:rows, k * P : (k + 1) * P],
                                rhs=ident[:rows, :rows],
                                start=(kk == 0),
                                stop=(kk == 3),
                            )
                        # split psum->sbuf copies across ACT and DVE
                        if half == 0:
                            nc.scalar.copy(out=xtb[:, 0 : 4 * P], in_=t_ps)
                        else:
                            nc.vector.tensor_copy(
                                out=xtb[:, 4 * P : 8 * P], in_=t_ps
                            )

                    # -- fused matmul: out = x_tile @ [Wc|W1|W2] --
                    out_ps = ps_out.tile([P, DOUT], f32, tag="out")
                    for j in range(2):
                        sl = slice(j * 512, (j + 1) * 512)
                        for k in range(KC):
                            nc.tensor.matmul(
                                out_ps[:rows, sl],
                                lhsT=xtb[:, k * P : k * P + rows],
                                rhs=w_sb[:, k, sl],
                                start=(k == 0),
                                stop=(k == KC - 1) if not with_bias else False,
                            )
                        if with_bias:
                            nc.tensor.matmul(
                                out_ps[:rows, sl],
                                lhsT=ones_row[0:1, :rows],
                                rhs=bias_sb[0:1, sl],
                                start=False,
                                stop=True,
                            )

                    if pending_agg is not None:
                        emit_agg(pending_agg)

                    # -- activations --
                    h_sb = acts_pool.tile([P, DC], bf16, tag="h")
                    nc.scalar.activation(
                        out=h_sb[:rows], in_=out_ps[:rows, 0:DC], func=af.Relu
                    )
                    t_sb = acts_pool.tile([P, H], bf16, tag="t")
                    nc.scalar.activation(
                        out=t_sb[:rows], in_=out_ps[:rows, DC : DC + H], func=af.Tanh
                    )
                    s_sb = acts_pool.tile([P, H], bf16, tag="s")
                    nc.scalar.activation(
                        out=s_sb[:rows],
                        in_=out_ps[:rows, DC + H : DOUT],
                        func=af.Sigmoid,
                    )
                    g_sb = acts_pool.tile([P, H], bf16, tag="g")
                    nc.vector.tensor_mul(g_sb[:rows], t_sb[:rows], s_sb[:rows])

                    # -- A[:, c] = sum_j g[:, j] * W3[j, c]  on DVE --
                    a_f32 = small_pool.tile([P, C], f32, tag="af32")
                    ttr_scr = small_pool.tile([P, H], f32, tag="ttrscr")
                    for c in range(C):
                        nc.vector.scalar_tensor_tensor(
                            out=ttr_scr[:rows],
                            in0=g_sb[:rows],
                            scalar=1.0,
                            in1=w3rep[:rows, c, :],
                            op0=mybir.AluOpType.mult,
                            op1=mybir.AluOpType.mult,
                            accum_out=a_f32[:rows, c : c + 1],
                        )

                    # -- stash A (both columns, f32) + bf16 copy for agg --
                    a_sb = small_pool.tile([P, C + 1], bf16, tag="asb")
                    nc.vector.tensor_copy(a_sb[:rows, 0:C], a_f32[:rows])
                    nc.vector.memset(a_sb[:rows, C : C + 1], 1.0)
                    nc.vector.tensor_copy(stage[:rows, t, :], a_f32[:rows])

                    pending_agg = (a_sb, h_sb, rows, t)

                emit_agg(pending_agg)

                # -- write outputs --
                agg_sb = small_pool.tile([C + 1, DC], f32, tag="aggsb")
                nc.vector.tensor_copy(agg_sb, agg_ps)
                nc.sync.dma_start(out=agg_ext.ap(), in_=agg_sb)
                nc.sync.dma_start(out=acol_ext.ap(), in_=stage)

            if n_iters > 1:
                with tc.For_i(0, n_iters, 1):
                    body()
            else:
                body()

    nc.compile()
    return nc


def _get_program(n_rows=NSH, n_iters=1, with_bias=False):
    key = (n_rows, n_iters, with_bias)
    if key not in _cache:
        _cache[key] = _build_program(n_rows, n_iters, with_bias)
    return _cache[key]


def _softmax(v):
    v = v - v.max()
    e = np.exp(v)
    return e / e.sum()


def run_device(x, wcat32, W3, bcat32, n_rows=NSH, n_cores=NCORES, n_iters=1):
    """Run the SPMD program; returns (agg_partials [cores,3,512],
    A_cols [cores, P, n_tiles, C]) as float64."""
    from concourse.bass_utils import run_bass_kernel_spmd

    with_bias = bool(np.any(bcat32))
    nc = _get_program(n_rows, n_iters, with_bias)
    wcat_b = wcat32.astype(ml_dtypes.bfloat16)
    w3_b = W3.astype(ml_dtypes.bfloat16)
    in_maps = []
    for i in range(n_cores):
        m = {
            "x": np.ascontiguousarray(x[i * n_rows : (i + 1) * n_rows]),
            "wcat": wcat_b,
            "w3": w3_b,
        }
        if with_bias:
            m["bcat"] = bcat32.reshape(1, DOUT)
        in_maps.append(m)
    res = run_bass_kernel_spmd(nc, in_maps, list(range(n_cores))).results
    agg = np.stack([r["agg"] for r in res]).astype(np.float64)
    acol = np.stack([r["acol"] for r in res]).astype(np.float64)
    return agg, acol


def kernel(**inputs):
    x = np.asarray(inputs["x"], dtype=np.float32)
    Wc = np.asarray(inputs["Wc"], dtype=np.float32)
    bc = np.asarray(inputs["bc"], dtype=np.float32)
    W1 = np.asarray(inputs["W1"], dtype=np.float32)
    b1 = np.asarray(inputs["b1"], dtype=np.float32)
    W2 = np.asarray(inputs["W2"], dtype=np.float32)
    b2 = np.asarray(inputs["b2"], dtype=np.float32)
    W3 = np.asarray(inputs["W3"], dtype=np.float32)
    b3 = np.asarray(inputs["b3"], dtype=np.float32)
    Wbag = np.asarray(inputs["Wbag"], dtype=np.float32)
    bbag = np.asarray(inputs["bbag"], dtype=np.float32)
    Wins = np.asarray(inputs["Wins"], dtype=np.float32)
    bins = np.asarray(inputs["bins"], dtype=np.float32)
    label = int(inputs["bag_label"])
    n_ins = int(inputs["n_ins"])

    wcat = np.concatenate([Wc, W1, W2], axis=1)          # [1024, 1024]
    bcat = np.concatenate([bc, b1, b2])                  # [1024]

    agg_p, acol_p = run_device(x, wcat, W3, bcat)

    # ---- slide head (host, exact given device agg partials) ----
    agg_all = agg_p.sum(axis=0)                          # [3, 512]
    slide_agg = agg_all[:C] + np.outer(b3.astype(np.float64), agg_all[C])
    score = slide_agg @ Wbag.astype(np.float64) + bbag.astype(np.float64)
    score = score.reshape(1, -1)                         # [1, 2]
    Y_prob = _softmax(score[0]).reshape(1, -1)
    Y_hat = np.int32(np.argmax(score[0]))

    # ---- assemble device A column over all N rows ----
    n_tiles = (NSH + P - 1) // P
    # acol_p[c, p, t, :] is row  c*NSH + t*128 + p
    a_dev = acol_p[:, :, :, label].transpose(0, 2, 1).reshape(NCORES, n_tiles * P)
    a_dev = a_dev[:, :NSH].reshape(-1)                   # [100000]

    # ---- shortlist + exact rescore in float64 ----
    K = min(N, max(64, 8 * n_ins))
    cand_pos = np.argpartition(-a_dev, K - 1)[:K]
    cand_neg = np.argpartition(a_dev, K - 1)[:K]
    cands = np.unique(np.concatenate([cand_pos, cand_neg]))
    xr = x[cands].astype(np.float64)
    a_ex = np.tanh(xr @ W1 + b1) * (1.0 / (1.0 + np.exp(-(xr @ W2 + b2))))
    A_ex = a_ex @ W3[:, label].astype(np.float64) + np.float64(b3[label])
    order = np.argsort(-A_ex, kind="stable")
    pos_idx = cands[order[:n_ins]]
    neg_idx = cands[order[::-1][:n_ins]]
    sel = np.concatenate([pos_idx, neg_idx])

    xs = x[sel].astype(np.float64)
    h_sel = np.maximum(xs @ Wc + bc, 0.0)
    logits = h_sel @ Wins + bins
    ins_logits = np.stack([_softmax(row) for row in logits]).astype(np.float32)
    ins_labels = np.concatenate(
        [np.ones(n_ins, np.int32), np.zeros(n_ins, np.int32)]
    )

    return (
        score.astype(np.float32),
        Y_hat,
        Y_prob.astype(np.float32),
        ins_logits,
        ins_labels,
    )
